# revision 6
# baseline (speedup 1.0000x reference)
"""Izhikevich spiking network (LiquidRON) Trainium2 Bass kernel.

Strategy: data-parallel over batch (B=64 -> 8 cores x 8). Each core runs the
full T=1000 sequential scan for its 8 batch rows.

Per-core layout: neurons padded 1000->1024 = 8 chunks x 128. State tiles are
[128 partitions = n-inner, 64 free = (chunk, batch)]. In this layout the
spike tile's column slice [:, 8k:8k+8] IS the matmul stationary for chunk k.

Matmul I = (0.5*S)^T-stream:  out[b, n] += sf[m,b]^T @ ST[m, n], bf16 hi+lo
split (products exact since sf in {0,1}; validated: reproduces the fp32
reference spike pattern exactly, |I err| ~1e-5 << min threshold margin 3e-4).
4-way PE column-tiling (tile_position=(0,32g)) gives 4 concurrent 256-wide
streams. The [8b x 256n] PSUM groups are transposed back to the state layout
with two fp32 PE transposes per step.

All elementwise math is fp32 IEEE on DVE, algebra arranged to be a bitwise
power-of-2 rescaling of the reference (u tracked as u/2, S,U,d pre-halved).
Resets use predicated copies => exact c / u+d values as in the reference.
"""
import contextlib
import os
import numpy as np
import ml_dtypes

T, B, N = int(os.environ.get("LIQ_T", "1000")), 64, 1000
NP = 1024          # padded neurons
CH = 8             # chunks of 128
BC = 8             # batches per core
NCORES = 8
W = 20             # steps per IO window
NW = T // W        # 50 windows
THRESH = 30.0
ALPHA = 0.14142135623730950488   # sqrt(0.02)
BETA = 12.374368670764581878     # 3.5 / (2*sqrt(0.02));  (a*v+b)^2 = .02v^2+3.5v+153.125
CONST = -83.125                  # 70 - 153.125

_cache = {}


def _build_program():
    import concourse.mybir as mybir
    from concourse import bass

    F32 = mybir.dt.float32
    BF16 = mybir.dt.bfloat16
    U8 = mybir.dt.uint8
    ge = mybir.AluOpType.is_ge
    add = mybir.AluOpType.add
    sub = mybir.AluOpType.subtract
    mult = mybir.AluOpType.mult

    nc = bass.Bass()
    sthi_d = nc.declare_dram_parameter("sthi", [128, CH * NP], BF16, isOutput=False)
    stlo_d = nc.declare_dram_parameter("stlo", [128, CH * NP], BF16, isOutput=False)
    cB_d = nc.declare_dram_parameter("cB", [128, 64], F32, isOutput=False)
    dB_d = nc.declare_dram_parameter("dB", [128, 64], F32, isOutput=False)
    A1_d = nc.declare_dram_parameter("A1", [128, 64], F32, isOutput=False)
    A2_d = nc.declare_dram_parameter("A2", [128, 64], F32, isOutput=False)
    id_d = nc.declare_dram_parameter("ident", [128, 128], F32, isOutput=False)
    data_d = nc.declare_dram_parameter("dataB", [NW, 128, W * 64], F32, isOutput=False)
    st_d = nc.declare_dram_parameter("statesO", [NW, 128, W * 64], U8, isOutput=True)
    v_d = nc.declare_dram_parameter("vO", [128, 64], F32, isOutput=True)
    u_d = nc.declare_dram_parameter("uO", [128, 64], F32, isOutput=True)

    es = contextlib.ExitStack()
    with es:
        block = es.enter_context(nc.Block())
        dsem = es.enter_context(nc.semaphore("dsem"))
        s_data = es.enter_context(nc.semaphore("s_data"))
        s_sf = es.enter_context(nc.semaphore("s_sf"))
        s_mm = es.enter_context(nc.semaphore("s_mm"))
        s_isb = es.enter_context(nc.semaphore("s_isb"))
        s_tr = es.enter_context(nc.semaphore("s_tr"))
        s_step = es.enter_context(nc.semaphore("s_step"))
        s_ost = es.enter_context(nc.semaphore("s_ost"))

        sthi = es.enter_context(nc.sbuf_tensor("sthi_sb", [128, CH * NP], BF16))
        stlo = es.enter_context(nc.sbuf_tensor("stlo_sb", [128, CH * NP], BF16))
        cB = es.enter_context(nc.sbuf_tensor("cB_sb", [128, 64], F32))
        dB = es.enter_context(nc.sbuf_tensor("dB_sb", [128, 64], F32))
        A1 = es.enter_context(nc.sbuf_tensor("A1_sb", [128, 64], F32))
        A2 = es.enter_context(nc.sbuf_tensor("A2_sb", [128, 64], F32))
        ident = es.enter_context(nc.sbuf_tensor("ident_sb", [128, 128], F32))
        dslot = [es.enter_context(nc.sbuf_tensor(f"dslot{i}", [128, W * 64], F32))
                 for i in range(2)]
        oslot = [es.enter_context(nc.sbuf_tensor(f"oslot{i}", [128, W * 64], U8))
                 for i in range(2)]
        v_s = es.enter_context(nc.sbuf_tensor("v_s", [128, 64], F32))
        u_s = es.enter_context(nc.sbuf_tensor("u_s", [128, 64], F32))
        sfb = [es.enter_context(nc.sbuf_tensor(f"sfb{i}", [128, 64], BF16))
               for i in range(2)]
        mask = es.enter_context(nc.sbuf_tensor("mask", [128, 64], mybir.dt.int32))
        ud_s = es.enter_context(nc.sbuf_tensor("ud_s", [128, 64], F32))
        t1_s = es.enter_context(nc.sbuf_tensor("t1_s", [128, 64], F32))
        sq_s = es.enter_context(nc.sbuf_tensor("sq_s", [128, 64], F32))
        z_s = es.enter_context(nc.sbuf_tensor("z_s", [128, 64], F32))
        w_s = es.enter_context(nc.sbuf_tensor("w_s", [128, 64], F32))
        m1_s = es.enter_context(nc.sbuf_tensor("m1_s", [128, 64], F32))
        m2_s = es.enter_context(nc.sbuf_tensor("m2_s", [128, 64], F32))
        isb = es.enter_context(nc.sbuf_tensor("isb", [128, 256], F32))
        p_i = es.enter_context(nc.psum_tensor("p_i", [128, 256], F32))
        p_t = es.enter_context(nc.psum_tensor("p_t", [128, 208], F32))

        NPRO = 7  # prologue loads (sthi stlo cB dB A1 A2 ident)

        @block.sync
        def _(sp: bass.BassEngine):
            for s, d in [(sthi, sthi_d), (stlo, stlo_d), (cB, cB_d), (dB, dB_d),
                         (A1, A1_d), (A2, A2_d), (ident, id_d)]:
                sp.dma_start(out=s.ap(), in_=d[:]).then_inc(dsem, 16)
            # first two data windows
            sp.dma_start(out=dslot[0].ap(), in_=data_d[0]).then_inc(s_data, 16)
            sp.dma_start(out=dslot[1].ap(), in_=data_d[1]).then_inc(s_data, 16)
            for i in range(NW):
                if i + 2 < NW:
                    # slot (i % 2) free once window i fully consumed
                    sp.wait_ge(s_step, W * i + W + 1)
                    sp.dma_start(out=dslot[i % 2].ap(), in_=data_d[i + 2]).then_inc(s_data, 16)
                # store states window i (filled through "step W*i+W")
                sp.wait_ge(s_step, W * i + W + 2)
                sp.dma_start(out=st_d[i], in_=oslot[i % 2].ap()).then_inc(s_ost, 16)
            sp.dma_start(out=v_d[:], in_=v_s.ap()).then_inc(s_ost, 16)
            sp.dma_start(out=u_d[:], in_=u_s.ap()).then_inc(s_ost, 16)
            sp.wait_ge(s_ost, 16 * NW + 32)

        @block.scalar
        def _(act: bass.BassScalarEngine):
            Copy = mybir.ActivationFunctionType.Copy
            for t in range(T):
                act.wait_ge(s_mm, t + 1)
                act.activation(isb.ap()[0:104, :], p_i.ap()[0:104, :], Copy
                               ).then_inc(s_isb, 1)

        @block.tensor
        def _(pe: bass.BassTensorEngine):
            pe.wait_ge(dsem, 16 * NPRO)
            for t in range(T):
                pe.wait_ge(s_sf, t + 1)
                for k in range(CH):
                    for half, st in ((0, sthi), (1, stlo)):
                        for g in range(4):
                            mm = pe.matmul(
                                p_i.ap()[32 * g:32 * g + BC, :],
                                sfb[t % 2].ap()[:, 8 * k:8 * k + 8],
                                st.ap()[:, NP * k + 256 * g: NP * k + 256 * g + 256],
                                start=(k == 0 and half == 0),
                                stop=(k == CH - 1 and half == 1),
                                tile_position=(0, 32 * g))
                            if k == CH - 1 and half == 1 and g == 3:
                                mm.then_inc(s_mm, 1)
                pe.wait_ge(s_isb, t + 1)
                pe.transpose(p_t.ap()[:, 0:104], isb.ap()[0:104, 0:128],
                             ident.ap()[0:104, 0:104])
                pe.transpose(p_t.ap()[:, 104:208], isb.ap()[0:104, 128:256],
                             ident.ap()[0:104, 0:104]).then_inc(s_tr, 1)

        @block.vector
        def _(vec: bass.BassVectorEngine):
            AP = bass.AP
            vec.wait_ge(dsem, 16 * NPRO)
            vec.memset(v_s.ap(), 0.0)
            vec.memset(u_s.ap(), 0.0)
            vec.nop().then_inc(s_step, 1)
            for t in range(T):
                i, s = t // W, t % W
                if s == 0:
                    vec.wait_ge(s_data, 16 * (i + 1))
                    if i >= 2:
                        vec.wait_ge(s_ost, 16 * (i - 1))
                # spike mask of current v == states[t-1]
                if t >= 1:
                    vec.wait_ge(s_tr, t)
                vec.tensor_scalar(mask.ap(), v_s.ap(), THRESH, None, ge)
                vec.tensor_scalar(sfb[t % 2].ap(), v_s.ap(), THRESH, None, ge
                                  ).then_inc(s_sf, 1)
                vec.wait_ge(s_mm, t + 1)
                if t >= 1:
                    io, so = (t - 1) // W, (t - 1) % W
                    vec.tensor_copy(oslot[io % 2].ap()[:, 64 * so:64 * so + 64],
                                    mask.ap())
                # resets (exact)
                vec.copy_predicated(v_s.ap(), mask.ap(), cB.ap())
                vec.tensor_tensor(ud_s.ap(), u_s.ap(), dB.ap(), add)
                vec.copy_predicated(u_s.ap(), mask.ap(), ud_s.ap())
                # quadratic
                vec.tensor_scalar(t1_s.ap(), v_s.ap(), ALPHA, BETA, mult, add)
                vec.tensor_tensor(sq_s.ap(), t1_s.ap(), t1_s.ap(), mult)
                # z = dataB + I  (even / odd chunks from transposed psum)
                vec.wait_ge(s_tr, t + 1)
                vec.tensor_tensor(
                    AP(z_s, 0, [[64, 128], [16, 4], [1, 8]]),
                    AP(dslot[i % 2], 64 * s, [[W * 64, 128], [16, 4], [1, 8]]),
                    AP(p_t, 0, [[208, 128], [32, 4], [1, 8]]), add)
                vec.tensor_tensor(
                    AP(z_s, 8, [[64, 128], [16, 4], [1, 8]]),
                    AP(dslot[i % 2], 64 * s + 8, [[W * 64, 128], [16, 4], [1, 8]]),
                    AP(p_t, 104, [[208, 128], [32, 4], [1, 8]]), add)
                # v' = sq + (z - u~)
                vec.tensor_tensor(w_s.ap(), z_s.ap(), u_s.ap(), sub)
                vec.tensor_tensor(v_s.ap(), sq_s.ap(), w_s.ap(), add)
                # u~' = (1-a) u~ + (ab/2) v'
                vec.tensor_tensor(m1_s.ap(), A1.ap(), u_s.ap(), mult)
                vec.tensor_tensor(m2_s.ap(), A2.ap(), v_s.ap(), mult)
                vec.tensor_tensor(u_s.ap(), m1_s.ap(), m2_s.ap(), add
                                  ).then_inc(s_step, 1)
            # final mask -> states[T-1]
            vec.tensor_scalar(mask.ap(), v_s.ap(), THRESH, None, ge)
            vec.tensor_copy(oslot[(NW - 1) % 2].ap()[:, 64 * (W - 1):64 * W],
                            mask.ap())
            vec.nop().then_inc(s_step, 1)
    return nc


def _prep_inputs(data, U, S, a, b, c, d):
    f32 = np.float32
    Np = NP
    # padded per-neuron params
    def pad(x, fill):
        out = np.full(Np, fill, f32)
        out[:N] = x
        return out
    a_p = pad(a, 0.0); b_p = pad(b, 0.0); c_p = pad(c, 0.0); d_p = pad(d, 0.0)
    A1v = (np.float32(1.0) - a_p)
    A2v = (a_p * b_p * np.float32(0.5))
    dHv = d_p * np.float32(0.5)

    def tile64(x):  # [NP] -> [128, 64] at (p, 8k+j) = x[128k+p]
        t = x.reshape(CH, 128).T            # [128, CH]
        return np.repeat(t[:, :, None], BC, axis=2).reshape(128, CH * BC).astype(f32)

    cB = tile64(c_p); dB = tile64(dHv); A1 = tile64(A1v); A2 = tile64(A2v)

    # S^T halves: ST[p, k*NP + n] = 0.5*S[n, 128k+p]
    Sh = np.zeros((Np, Np), f32)
    Sh[:N, :N] = np.float32(0.5) * S
    STf = Sh.T.reshape(CH, 128, Np)          # [k, p, n] = Sh[128k+p, n]^T? careful:
    # Sh.T[m, n] = Sh[n, m] -> element [m, n]; reshape m -> (k, p)
    hi = STf.astype(ml_dtypes.bfloat16)
    lo = (STf - hi.astype(f32)).astype(ml_dtypes.bfloat16)
    sthi = np.ascontiguousarray(hi.transpose(1, 0, 2).reshape(128, CH * Np))
    stlo = np.ascontiguousarray(lo.transpose(1, 0, 2).reshape(128, CH * Np))

    ident = np.eye(128, dtype=f32)

    # dataB per core: [NW, 128, W*64]; value (i, p, 64s + 8k + j) =
    #   0.5*U[n]*data[20i+s, b0+j, n] + CONST,  n = 128k+p  (pad n: CONST)
    Uh = np.zeros(Np, f32)
    Uh[:N] = np.float32(0.5) * U
    dataBs = []
    for core in range(NCORES):
        b0 = core * BC
        dc = data[:, b0:b0 + BC, :]                       # [T, 8, N]
        x = np.full((T, BC, Np), 0.0, f32)
        x[:, :, :N] = dc
        x = x * Uh + np.float32(CONST)                    # [T, 8, NP]
        x = x.reshape(T, BC, CH, 128)                     # [t, j, k, p]
        x = x.transpose(0, 3, 2, 1)                       # [t, p, k, j]
        x = x.reshape(NW, W, 128, CH * BC)                # [i, s, p, 64]
        x = x.transpose(0, 2, 1, 3)                       # [i, p, s, 64]
        dataBs.append(np.ascontiguousarray(x.reshape(NW, 128, W * 64)))

    shared = {"sthi": sthi, "stlo": stlo, "cB": cB, "dB": dB,
              "A1": A1, "A2": A2, "ident": ident}
    return [dict(shared, dataB=dataBs[core]) for core in range(NCORES)]


def kernel(data, U, S, a, b, c, d):
    from concourse.bass_utils import run_bass_kernel_spmd

    data = np.asarray(data, np.float32)
    in_maps = _prep_inputs(data, np.asarray(U, np.float32),
                           np.asarray(S, np.float32), np.asarray(a, np.float32),
                           np.asarray(b, np.float32), np.asarray(c, np.float32),
                           np.asarray(d, np.float32))
    if "nc" not in _cache:
        _cache["nc"] = _build_program()
    res = run_bass_kernel_spmd(_cache["nc"], in_maps, list(range(NCORES))).results

    states = np.empty((T, B, N), np.bool_)
    v = np.empty((B, N), np.float32)
    u = np.empty((B, N), np.float32)
    for core in range(NCORES):
        b0 = core * BC
        raw = res[core]["statesO"].reshape(NW, 128, W, 64)   # [i, p, s, 64]
        raw = raw.transpose(0, 2, 1, 3).reshape(T, 128, CH, BC)  # [t, p, k, j]
        st = raw.transpose(0, 3, 2, 1).reshape(T, BC, NP)[:, :, :N]
        states[:, b0:b0 + BC, :] = st.astype(np.bool_)
        vO = res[core]["vO"].reshape(128, CH, BC)
        uO = res[core]["uO"].reshape(128, CH, BC)
        v[b0:b0 + BC, :] = vO.transpose(2, 1, 0).reshape(BC, NP)[:, :N]
        u[b0:b0 + BC, :] = np.float32(2.0) * uO.transpose(2, 1, 0).reshape(BC, NP)[:, :N]

    spikes = np.zeros((T, B, N), np.float32)
    spikes[1:] = states[:-1]
    return states, v, u, spikes
